# revision 37
# baseline (speedup 1.0000x reference)
"""Trainium2 Bass kernel for nn_ContrastiveLoss (B=4096, F=256, T=0.1).

Strategy (8 NeuronCores, symmetric "circulant cover" of the 8192x8192
similarity matrix -- each unordered pair (i,j) computed exactly once):

  - Host rolls the combined [2B, F] matrix by 128*k rows for core k. The
    device program is IDENTICAL on all cores: it owns canonical row-tiles
    r in {0,8,...,56} and computes the cyclic band sim block rows
    [128r, 128r+128) x cols [128r, 128r + 128w) (mod 8192), w = 33 for
    r < 32 and 32 for r >= 32. Unioned over the 8 rotations this covers
    every unordered pair exactly once (total work is HALF the full gram).
  - All 64 row-tiles are normalized in bf16 (fp32->bf16 convert on GpSimd,
    sum-of-squares + rsqrt Newton + scale on DVE at 2x rate), transposed
    via DMA xbar into cT [256, 8192] bf16.
  - Per column-group (2048 cols, ascending), each strip's block is matmul'd
    (bf16, 2 contraction chunks) into PSUM, exp'd on ACT (scale=1/T) with
    fused row-sum accumulation.  exp output: bf16 for diag-containing
    chunks, fp8e4 otherwise (off-diag |s|<=~0.5 so exp(s/T) < 240).
  - Column sums of the exp blocks (the "lower triangle" contributions) are
    computed on the PE with ones-vector matmuls into per-512-col PSUM
    accumulators; fp8 chunks with identical column ranges are paired into
    DoubleRow matmuls (2 strips per pass).
  - gf = sum_j c_j (for the rank-1 raw-sum identity sum_ij s_ij = |gf|^2)
    via tiny ones matmuls over own scaled tiles; pos-pair dots and
    d_i = |c_i|^2 via DVE STT with accumulate on own scaled tiles.
  - Host finishes in float64: E_i = rowsum + colsums - exp(10 d_i),
    loss = -mean(pos)/T + ((|gf|^2 - sum d)/T - (2B-1) sum log E)/4B^2.
"""

import sys

sys.path.insert(0, "/opt/trn_rl_repo")

from contextlib import ExitStack  # noqa: E402

import numpy as np  # noqa: E402

import concourse.bass as bass  # noqa: E402
import concourse.mybir as mybir  # noqa: E402
import concourse.tile as tile  # noqa: E402
from concourse import bacc  # noqa: E402
from concourse.bass_utils import run_bass_kernel_spmd  # noqa: E402

B = 4096
F = 256
TWO_B = 2 * B
N_CORES = 8
INV_T = 10.0
EPS2 = 1e-14

F32 = mybir.dt.float32
BF16 = mybir.dt.bfloat16
FP8 = mybir.dt.float8e4
U32 = mybir.dt.uint32
OP = mybir.AluOpType
AX = mybir.AxisListType

NT = 64  # 128-row tiles of the combined matrix
OWN = [0, 8, 16, 24, 32, 40, 48, 56]  # canonical own row-tiles
BANDW = {r: (33 if r < 32 else 32) for r in OWN}

# stats tile layout (columns)
S_E = 0  # 0:8   exp row-sums per own tile (incl. diagonal term)
S_D = 8  # 8:16  d_i = ||c_i||^2 for own rows
S_POS = 16  # 16:20 positive-pair dot sums (4 pairs)
S_GF = 20  # 20:22 column sums of own scaled rows (per K-chunk)
S_W = 24

CHUNK = 1024  # psum ping-pong half width (2 banks)


def _work_items():
    """Static work list: per column-group G (ascending), the list of
    (own-index m, col0, col1, diag) block items, fp8 items greedily paired
    for DoubleRow column-sum matmuls.

    Returns list of groups; each group is a list of "emission units":
      ("single", m, c0, c1, diag)     one strip chunk (bf16 et if diag)
      ("pair", m1, m2, c0, c1)        two fp8 strip chunks, DR colsum
    """
    groups = []
    for G in range(4):
        g0, g1 = 2048 * G, 2048 * (G + 1)
        items = []  # (m, c0, c1, diag)
        for m, r in enumerate(OWN):
            start, width = 128 * r, 128 * BANDW[r]
            for s0, s1 in (
                (start, min(start + width, TWO_B)),
                (0, max(0, start + width - TWO_B)),
            ):
                c0, c1 = max(s0, g0), min(s1, g1)
                if c0 >= c1:
                    continue
                # chop into <= CHUNK pieces
                c = c0
                while c < c1:
                    ce = min(c + CHUNK, c1)
                    items.append((m, c, ce, c == start))
                    c = ce
        # greedy pairing of non-diag items with identical ranges
        units, unpaired = [], {}
        for it in items:
            m, c0, c1, diag = it
            if diag:
                units.append(("single", m, c0, c1, True))
            elif (c0, c1) in unpaired:
                m1 = unpaired.pop((c0, c1))
                units.append(("pair", m1, m, c0, c1))
            else:
                unpaired[(c0, c1)] = m
        for (c0, c1), m in unpaired.items():
            units.append(("single", m, c0, c1, False))
        groups.append(units)
    return groups


WORK = _work_items()


def _build_kernel(loop_n=None):
    nc = bacc.Bacc("TRN2", target_bir_lowering=False, debug=False, num_devices=N_CORES)

    cmb = nc.dram_tensor("cmb", [TWO_B, F], F32, kind="ExternalInput")
    seg_in = nc.dram_tensor("seg_in", [128, 32], F32, kind="ExternalInput")
    out = nc.dram_tensor("out", [128, S_W], F32, kind="ExternalOutput")
    ocols = nc.dram_tensor("ocols", [4, 32, 2048], F32, kind="ExternalOutput")

    with tile.TileContext(nc) as tc, ExitStack() as octx:
        if loop_n is not None:
            octx.enter_context(tc.For_i(0, loop_n, 1))
        _emit_body(nc, tc, cmb, seg_in, out, ocols)

    nc.compile()
    return nc


def _emit_body(nc, tc, cmb, seg_in, out, ocols):
    with ExitStack() as ctx:
        singles = ctx.enter_context(tc.tile_pool(name="singles", bufs=1))
        rawp = ctx.enter_context(tc.tile_pool(name="rawp", bufs=8))
        scr = ctx.enter_context(tc.tile_pool(name="scr", bufs=4))
        psum = ctx.enter_context(tc.tile_pool(name="psum", bufs=1, space="PSUM"))
        colp = ctx.enter_context(tc.tile_pool(name="colp", bufs=1, space="PSUM"))
        etpool = ctx.enter_context(tc.tile_pool(name="etpool", bufs=4))

        stats = singles.tile([128, S_W], F32)
        raw_own = singles.tile([128, 8, F], F32)
        scaled_sb = [
            singles.tile([128, 2, 8, 128], BF16, name=f"scsb{i}") for i in range(8)
        ]  # chunk-major normalized, one memref per 8-tile sub-batch
        cT = [
            [singles.tile([128, 2048], BF16, name=f"cT{G}_{c}") for c in range(2)]
            for G in range(4)
        ]
        blkT = [singles.tile([128, 8 * 128], BF16, name=f"blkT{c}") for c in range(2)]
        ssb = [singles.tile([128, 8], F32, name=f"ssb{i}") for i in range(8)]
        yB = [singles.tile([128, 8], F32, name=f"yB{i}") for i in range(8)]
        ybb = [singles.tile([128, 8], BF16, name=f"ybb{i}") for i in range(8)]
        e_parts = singles.tile([128, 8, 20], F32)
        ones_b = singles.tile([128, 2], BF16)
        seg = singles.tile([128, 32], BF16)  # seg[p, g] = 1 iff p//4 == g
        seg8 = singles.tile([128, 2, 32], FP8)  # doubled for DR colsums
        magicf = singles.tile([128, NT], F32)
        ostage = ctx.enter_context(tc.tile_pool(name='ostage', bufs=2))

        psum_half = [
            psum.tile([128, CHUNK], F32, name=f"psh{i}") for i in range(2)
        ]  # 2+2 banks, ping-pong as separate memrefs (finer dep tracking)

        nc.vector.memset(magicf[:], float(0x5F3759DF))
        nc.vector.memset(ones_b[:], 1.0)
        segf = singles.tile([128, 32], F32)
        nc.sync.dma_start(segf[:], seg_in.ap())
        nc.vector.tensor_copy(seg[:], segf[:])
        for i in range(2):
            nc.vector.tensor_copy(seg8[:, i, :], seg[:])
        nc.vector.memset(e_parts[:], 0.0)

        cmb_t = cmb.ap().rearrange("(t p) f -> p t f", p=128)

        # ---- DMA loads: own tiles first, then 8-tile sub-batches ascending -
        # (transposes are interleaved into the same SP ring later, in
        # consumption order, to avoid head-of-line blocking)
        for m, r in enumerate(OWN):
            nc.sync.dma_start(raw_own[:, m, :], cmb_t[:, r, :])
        raw_sb = []

        def load_sb(sb):
            rg = rawp.tile([128, 8, F], F32, tag="rawsb")
            nc.sync.dma_start(rg[:], cmb_t[:, 8 * sb : 8 * (sb + 1), :])
            raw_sb.append(rg)

        for sb in range(8):
            load_sb(sb)

        # ---- normalization helpers -----------------------------------------
        def sumsq(raw_view, ss_col, eng=None):
            sq = scr.tile([128, F], BF16, tag="sq")
            (eng or nc.vector).scalar_tensor_tensor(
                out=sq[:], in0=raw_view, scalar=0.0, in1=raw_view,
                op0=OP.bypass, op1=OP.mult, accum_out=ss_col,
            )

        def newton(sb):
            """rsqrt of ssb[sb] -> yB[sb] (fp32) and ybb[sb] (bf16)."""
            ss, y = ssb[sb], yB[sb]
            n = 8
            sm = scr.tile([128, n], F32, tag="nm")
            nc.vector.tensor_scalar_max(sm[:], ss[:], EPS2)
            bits_f = scr.tile([128, n], F32, tag="nb")
            nc.vector.tensor_copy(bits_f[:], sm[:].bitcast(U32))
            seed_f = scr.tile([128, n], F32, tag="ns")
            nc.vector.scalar_tensor_tensor(
                out=seed_f[:], in0=bits_f[:], scalar=-0.5, in1=magicf[:, :n],
                op0=OP.mult, op1=OP.add,
            )
            nc.vector.tensor_copy(y[:].bitcast(U32), seed_f[:])
            for _ in range(2):
                t1 = scr.tile([128, n], F32, tag="nr")
                nc.vector.tensor_tensor(t1[:], y[:], y[:], OP.mult)
                t2 = scr.tile([128, n], F32, tag="nr")
                nc.vector.scalar_tensor_tensor(
                    out=t2[:], in0=t1[:], scalar=-0.5, in1=sm[:],
                    op0=OP.mult, op1=OP.mult,
                )
                t3 = scr.tile([128, n], F32, tag="nr")
                nc.vector.tensor_scalar_add(t3[:], t2[:], 1.5)
                nc.vector.tensor_tensor(y[:], y[:], t3[:], OP.mult)
            nc.vector.tensor_copy(ybb[sb][:], y[:])

        def scale_tiles(raw_view, sb):
            """scaled_sb[sb] = raw * yb[t] (bf16, on GpSimd: DVE is the
            pacing engine and the Q7s are otherwise idle)."""
            yv = ybb[sb][:].broadcast_to([128, 8, 128])
            for c in range(2):
                nc.vector.scalar_tensor_tensor(
                    out=scaled_sb[sb][:, c, :, :],
                    in0=raw_view[:, :, 128 * c : 128 * (c + 1)],
                    scalar=1.0, in1=yv, op0=OP.mult, op1=OP.mult,
                )

        # ---- own tiles: normalize, transpose, gf --------------------------
        ss_own = singles.tile([128, 8], F32)
        for m in range(8):
            sumsq(raw_own[:, m, :], ss_own[:, m : m + 1])
        # own norms -> compact newton, then scatter scale per tile
        y_own = singles.tile([128, 8], F32)
        yb_own = singles.tile([128, 8], BF16)
        nc.vector.tensor_scalar_max(ss_own[:], ss_own[:], EPS2)
        bits_f = scr.tile([128, 8], F32, tag="nb")
        nc.vector.tensor_copy(bits_f[:], ss_own[:].bitcast(U32))
        seed_f = scr.tile([128, 8], F32, tag="ns")
        nc.vector.scalar_tensor_tensor(
            out=seed_f[:], in0=bits_f[:], scalar=-0.5, in1=magicf[:, :8],
            op0=OP.mult, op1=OP.add,
        )
        nc.vector.tensor_copy(y_own[:].bitcast(U32), seed_f[:])
        for _ in range(2):
            t1 = scr.tile([128, 8], F32, tag="nr")
            nc.vector.tensor_tensor(t1[:], y_own[:], y_own[:], OP.mult)
            t2 = scr.tile([128, 8], F32, tag="nr")
            nc.vector.scalar_tensor_tensor(
                out=t2[:], in0=t1[:], scalar=-0.5, in1=ss_own[:],
                op0=OP.mult, op1=OP.mult,
            )
            t3 = scr.tile([128, 8], F32, tag="nr")
            nc.vector.tensor_scalar_add(t3[:], t2[:], 1.5)
            nc.vector.tensor_tensor(y_own[:], y_own[:], t3[:], OP.mult)
        nc.vector.tensor_copy(yb_own[:], y_own[:])
        scaled_own = singles.tile([128, 2, 8, 128], BF16)
        yv_own = yb_own[:].broadcast_to([128, 8, 128])
        for c in range(2):
            nc.vector.scalar_tensor_tensor(
                out=scaled_own[:, c, :, :],
                in0=raw_own[:, :, 128 * c : 128 * (c + 1)],
                scalar=1.0, in1=yv_own, op0=OP.mult, op1=OP.mult,
            )
        # seed ss with the own sums-of-squares so each sub-batch's newton and
        # scale cover the own tiles too (bit-identical to the own-phase path)
        for m, r in enumerate(OWN):
            nc.vector.tensor_copy(ssb[r // 8][:, r % 8 : r % 8 + 1],
                                  ss_own[:, m : m + 1])
        # own transposes into blkT (batched: one DMA per K-chunk)
        for c in range(2):
            nc.sync.dma_start_transpose(
                out=blkT[c][:].rearrange("p (t i) -> p t i", i=128),
                in_=scaled_own[:, c, :, :],
            )
        # gf: sum of own scaled rows per K-chunk (tiny ones matmuls)
        for c in range(2):
            for m in range(8):
                nc.tensor.matmul(
                    psum_half[0][:, c : c + 1],
                    scaled_own[:, c, m, :],
                    ones_b[:, 0:1],
                    start=(m == 0), stop=(m == 7),
                )
        nc.vector.tensor_copy(stats[:, S_GF : S_GF + 2], psum_half[0][:, 0:2])
        # ---- normalization + transposes, 8-tile sub-batches (ascending) ----
        for sb in range(8):
            t0 = 8 * sb
            rg = raw_sb[sb]
            for i in range(8):
                t = t0 + i
                if t in OWN:
                    continue
                sumsq(rg[:, i, :], ssb[sb][:, i : i + 1])
            with tc.high_priority(offset=40):
                newton(sb)
                scale_tiles(rg[:], sb)
                for c in range(2):
                    nc.sync.dma_start_transpose(
                        out=cT[sb // 2][c][
                            :, 1024 * (sb % 2) : 1024 * (sb % 2 + 1)
                        ].rearrange("p (t i) -> p t i", i=128),
                        in_=scaled_sb[sb][:, c, :, :],
                    )


        # ---- main loop: per column-group, strips -> exp -> colsums ---------
        half = 0
        eslot = [0] * 8  # rowsum slot counter per own tile
        pending = []  # deferred colsum emissions (1-unit lag for PE overlap)

        def emit_mms(m, c0, c1, ph):
            w = c1 - c0
            G, gof = c0 // 2048, c0 % 2048
            for c in range(2):
                lhs = blkT[c][:, 128 * m : 128 * (m + 1)]
                o = 0
                while o < w:
                    oe = min(o + 512, w)
                    nc.tensor.matmul(
                        psum_half[ph][:, o:oe],
                        lhs, cT[G][c][:, gof + o : gof + oe],
                        start=(c == 0), stop=(c == 1),
                    )
                    o = oe

        def flush_pending():
            while pending:
                pending.pop(0)()

        colacc = None
        colacc_first = None

        for G in range(4):
            colacc = colp.tile([128, 2048], F32, tag="colacc")
            colacc_first = [True] * 4  # per 512-chunk of this group

            def colsum_mm(et_ap, c0, c1, dr):
                """Emit colsum matmuls for et (SBUF) covering cols [c0,c1)."""
                c = c0
                while c < c1:
                    q, qoff = divmod(c - 2048 * G, 512)
                    ce = min(c1, 2048 * G + 512 * (q + 1))
                    first = colacc_first[q]
                    colacc_first[q] = False
                    out_ap = colacc[0:32, 512 * q + qoff : 512 * q + qoff + (ce - c)]
                    in_ap = et_ap[:, c - c0 : ce - c0] if not dr else \
                        et_ap[:, :, c - c0 : ce - c0]
                    nc.tensor.matmul(
                        out_ap,
                        seg8[:] if dr else seg[:],
                        in_ap,
                        start=first, stop=False,
                        perf_mode=mybir.MatmulPerfMode.DoubleRow if dr else None,
                        skip_group_check=True,
                    )
                    c = ce

            for unit in WORK[G]:
                if unit[0] == "single":
                    _, m, c0, c1, diag = unit
                    w = c1 - c0
                    ph = half
                    half ^= 1
                    emit_mms(m, c0, c1, ph)
                    et = etpool.tile([128, CHUNK], BF16, tag="etb")
                    sl = eslot[m]
                    eslot[m] += 1
                    nc.scalar.activation(
                        et[:, :w], psum_half[ph][:, :w],
                        mybir.ActivationFunctionType.Exp,
                        bias=0.0, scale=INV_T,
                        accum_out=e_parts[:, m, sl : sl + 1],
                    )
                    cs0 = c0 + 128 if diag else c0
                    if cs0 < c1:
                        pending.append(
                            lambda et=et, cs0=cs0, c0=c0, c1=c1: colsum_mm(
                                et[:, cs0 - c0 :], cs0, c1, False
                            )
                        )
                else:
                    _, m1, m2, c0, c1 = unit
                    w = c1 - c0
                    etp = etpool.tile([128, 2, CHUNK], FP8, tag="etp")
                    for pi, m in enumerate((m1, m2)):
                        ph = half
                        half ^= 1
                        emit_mms(m, c0, c1, ph)
                        sl = eslot[m]
                        eslot[m] += 1
                        nc.scalar.activation(
                            etp[:, pi, :w], psum_half[ph][:, :w],
                            mybir.ActivationFunctionType.Exp,
                            bias=0.0, scale=INV_T,
                            accum_out=e_parts[:, m, sl : sl + 1],
                        )
                    pending.append(
                        lambda etp=etp, c0=c0, c1=c1: colsum_mm(
                            etp[:, :, : c1 - c0], c0, c1, True
                        )
                    )
                # lagged colsum emission: keep PE two units ahead so the
                # colsum matmul (which waits on an earlier exp) never blocks
                # the next unit's matmuls in PE program order
                while len(pending) > 2:
                    pending.pop(0)()
            flush_pending()
            # drain this group's colsum accumulators to SBUF, then to DRAM
            stg = ostage.tile([128, 2048], F32, tag="ostg")
            nc.vector.tensor_copy(stg[0:32, :], colacc[0:32, :])
            nc.sync.dma_start(ocols.ap()[G, :, :], stg[0:32, :])

        # d_i and pos-pair dots from own scaled tiles
        for m in range(8):
            sq = scr.tile([128, 2, 128], BF16, tag="sqd")
            nc.vector.scalar_tensor_tensor(
                out=sq[:], in0=scaled_own[:, :, m, :], scalar=0.0,
                in1=scaled_own[:, :, m, :], op0=OP.bypass, op1=OP.mult,
                accum_out=stats[:, S_D + m : S_D + m + 1],
            )
        for a in range(4):
            sq = scr.tile([128, 2, 128], BF16, tag="sqp")
            nc.vector.scalar_tensor_tensor(
                out=sq[:], in0=scaled_own[:, :, a, :], scalar=0.0,
                in1=scaled_own[:, :, a + 4, :], op0=OP.bypass, op1=OP.mult,
                accum_out=stats[:, S_POS + a : S_POS + a + 1],
            )

        # ---- finalize stats -------------------------------------------------
        for m in range(8):
            nc.vector.tensor_reduce(
                stats[:, S_E + m : S_E + m + 1], e_parts[:, m, :], AX.X, OP.add
            )
        nc.sync.dma_start(out.ap(), stats[:])


_NC_CACHE = None


def _get_nc():
    global _NC_CACHE
    if _NC_CACHE is None:
        _NC_CACHE = _build_kernel()
    return _NC_CACHE


def make_in_maps(first, second):
    f = np.ascontiguousarray(first, dtype=np.float32)
    s = np.ascontiguousarray(second, dtype=np.float32)
    cmb = np.concatenate([f, s], axis=0)
    seg_host = np.zeros((128, 32), dtype=np.float32)
    seg_host[np.arange(128), np.arange(128) // 4] = 1.0
    return [
        {
            "cmb": np.ascontiguousarray(np.roll(cmb, -128 * k, axis=0)),
            "seg_in": seg_host,
        }
        for k in range(N_CORES)
    ]


def combine_outputs(outs):
    """outs: list of 8 dicts with 'out' [128, S_W] and 'ocols' [4, 8192]."""
    E = np.zeros(TWO_B, dtype=np.float64)  # global-row exp sums
    d = np.zeros(TWO_B, dtype=np.float64)
    gf = np.zeros((128, 2), dtype=np.float64)
    pos_tot = 0.0
    for k in range(N_CORES):
        st = np.asarray(outs[k]["out"], dtype=np.float64)
        oc = np.asarray(outs[k]["ocols"], dtype=np.float64)
        # rowsums + d for own canonical tiles -> global rows
        for m, r in enumerate(OWN):
            rows = (np.arange(128) + 128 * (r + k)) % TWO_B  # canonical->global
            E[rows] += st[:, S_E + m]
            d[rows] = st[:, S_D + m]
        pos_tot += st[:, S_POS : S_POS + 4].sum()
        gf += st[:, S_GF : S_GF + 2]
        # colsums: oc[G, g, n] = partial sum (rows 4g..4g+4) for canonical
        # col 2048G + n
        cc = oc.sum(axis=1)  # [G, n]
        can_cols = 2048 * np.arange(4)[:, None] + np.arange(2048)[None, :]
        gcols = (can_cols + 128 * k) % TWO_B
        np.add.at(E, gcols.ravel(), cc.ravel())
    E_excl = E - np.exp(INV_T * d)
    lse = np.log(E_excl)
    raw_excl = (float((gf * gf).sum()) - d.sum()) * INV_T
    neg = raw_excl - (TWO_B - 1) * lse.sum()
    loss = -pos_tot * INV_T / B + neg / (4.0 * B * B)
    return np.asarray(loss, dtype=np.float32)


def kernel(first_transformed, second_transformed):
    nc = _get_nc()
    in_maps = make_in_maps(first_transformed, second_transformed)
    res = run_bass_kernel_spmd(nc, in_maps, core_ids=list(range(N_CORES)))
    return combine_outputs(res.results)


# revision 40
# speedup vs baseline: 1.0486x; 1.0486x over previous
"""Trainium2 Bass kernel for nn_ContrastiveLoss (B=4096, F=256, T=0.1).

Strategy (8 NeuronCores, symmetric "circulant cover" of the 8192x8192
similarity matrix -- each unordered pair (i,j) computed exactly once):

  - Host rolls the combined [2B, F] matrix by 128*k rows for core k. The
    device program is IDENTICAL on all cores: it owns canonical row-tiles
    r in {0,8,...,56} and computes the cyclic band sim block rows
    [128r, 128r+128) x cols [128r, 128r + 128w) (mod 8192), w = 33 for
    r < 32 and 32 for r >= 32. Unioned over the 8 rotations this covers
    every unordered pair exactly once (total work is HALF the full gram).
  - All 64 row-tiles are normalized in bf16 (fp32->bf16 convert on GpSimd,
    sum-of-squares + rsqrt Newton + scale on DVE at 2x rate), transposed
    via DMA xbar into cT [256, 8192] bf16.
  - Per column-group (2048 cols, ascending), each strip's block is matmul'd
    (bf16, 2 contraction chunks) into PSUM, exp'd on ACT (scale=1/T) with
    fused row-sum accumulation.  exp output: bf16 for diag-containing
    chunks, fp8e4 otherwise (off-diag |s|<=~0.5 so exp(s/T) < 240).
  - Column sums of the exp blocks (the "lower triangle" contributions) are
    computed on the PE with ones-vector matmuls into per-512-col PSUM
    accumulators; fp8 chunks with identical column ranges are paired into
    DoubleRow matmuls (2 strips per pass).
  - gf = sum_j c_j (for the rank-1 raw-sum identity sum_ij s_ij = |gf|^2)
    via tiny ones matmuls over own scaled tiles; pos-pair dots and
    d_i = |c_i|^2 via DVE STT with accumulate on own scaled tiles.
  - Host finishes in float64: E_i = rowsum + colsums - exp(10 d_i),
    loss = -mean(pos)/T + ((|gf|^2 - sum d)/T - (2B-1) sum log E)/4B^2.
"""

import sys

sys.path.insert(0, "/opt/trn_rl_repo")

from contextlib import ExitStack  # noqa: E402

import numpy as np  # noqa: E402

import concourse.bass as bass  # noqa: E402
import concourse.mybir as mybir  # noqa: E402
import concourse.tile as tile  # noqa: E402
from concourse import bacc  # noqa: E402
from concourse.bass_utils import run_bass_kernel_spmd  # noqa: E402

B = 4096
F = 256
TWO_B = 2 * B
N_CORES = 8
INV_T = 10.0
EPS2 = 1e-14

F32 = mybir.dt.float32
BF16 = mybir.dt.bfloat16
FP8 = mybir.dt.float8e4
U32 = mybir.dt.uint32
OP = mybir.AluOpType
AX = mybir.AxisListType

NT = 64  # 128-row tiles of the combined matrix
OWN = [0, 8, 16, 24, 32, 40, 48, 56]  # canonical own row-tiles
BANDW = {r: (33 if r < 32 else 32) for r in OWN}

# stats tile layout (columns)
S_E = 0  # 0:8   exp row-sums per own tile (incl. diagonal term)
S_D = 8  # 8:16  d_i = ||c_i||^2 for own rows
S_POS = 16  # 16:20 positive-pair dot sums (4 pairs)
S_GF = 20  # 20:22 column sums of own scaled rows (per K-chunk)
S_W = 24

CHUNK = 1024  # psum ping-pong half width (2 banks)

# Schraudolph fast-exp constants (DVE offload): exp(10*s) ~= bitcast_f32(
# int(A*s + B)).  fp32 values >= 2^23 are integral, so the int conversion is
# exact; B is calibrated offline so the exp-weighted mean error is ~4e-6.
SCHRAUD_A = 121031472.0
SCHRAUD_B = 1064870656.0
# pairs per column-group converted to DVE fast-exp (late groups: DVE is idle
# there while ACT is the bottleneck)
DVE_EXP_QUOTA = {0: 0, 1: 0, 2: 0, 3: 0}


def _work_items():
    """Static work list: per column-group G (ascending), the list of
    (own-index m, col0, col1, diag) block items, fp8 items greedily paired
    for DoubleRow column-sum matmuls.

    Returns list of groups; each group is a list of "emission units":
      ("single", m, c0, c1, diag)     one strip chunk (bf16 et if diag)
      ("pair", m1, m2, c0, c1)        two fp8 strip chunks, DR colsum
    """
    groups = []
    for G in range(4):
        g0, g1 = 2048 * G, 2048 * (G + 1)
        items = []  # (m, c0, c1, diag)
        for m, r in enumerate(OWN):
            start, width = 128 * r, 128 * BANDW[r]
            for s0, s1 in (
                (start, min(start + width, TWO_B)),
                (0, max(0, start + width - TWO_B)),
            ):
                c0, c1 = max(s0, g0), min(s1, g1)
                if c0 >= c1:
                    continue
                # chop into <= CHUNK pieces
                c = c0
                while c < c1:
                    ce = min(c + CHUNK, c1)
                    items.append((m, c, ce, c == start))
                    c = ce
        # greedy pairing of non-diag items with identical ranges
        units, unpaired = [], {}
        for it in items:
            m, c0, c1, diag = it
            if diag:
                units.append(("single", m, c0, c1, True))
            elif (c0, c1) in unpaired:
                m1 = unpaired.pop((c0, c1))
                units.append(("pair", m1, m, c0, c1))
            else:
                unpaired[(c0, c1)] = m
        for (c0, c1), m in unpaired.items():
            units.append(("single", m, c0, c1, False))
        quota = DVE_EXP_QUOTA[G]
        for i in range(len(units) - 1, -1, -1):
            if quota <= 0:
                break
            if units[i][0] == "pair":
                units[i] = ("dpair",) + units[i][1:]
                quota -= 1
        groups.append(units)
    return groups


WORK = _work_items()


def _build_kernel(loop_n=None):
    nc = bacc.Bacc("TRN2", target_bir_lowering=False, debug=False, num_devices=N_CORES)

    cmb = nc.dram_tensor("cmb", [TWO_B, F], F32, kind="ExternalInput")
    seg_in = nc.dram_tensor("seg_in", [128, 32], F32, kind="ExternalInput")
    out = nc.dram_tensor("out", [128, S_W], F32, kind="ExternalOutput")
    ocols = nc.dram_tensor("ocols", [4, 32, 2048], F32, kind="ExternalOutput")

    with tile.TileContext(nc) as tc, ExitStack() as octx:
        if loop_n is not None:
            octx.enter_context(tc.For_i(0, loop_n, 1))
        _emit_body(nc, tc, cmb, seg_in, out, ocols)

    nc.compile()
    return nc


def _emit_body(nc, tc, cmb, seg_in, out, ocols):
    with ExitStack() as ctx:
        singles = ctx.enter_context(tc.tile_pool(name="singles", bufs=1))
        rawp = ctx.enter_context(tc.tile_pool(name="rawp", bufs=8))
        scr = ctx.enter_context(tc.tile_pool(name="scr", bufs=4))
        psum = ctx.enter_context(tc.tile_pool(name="psum", bufs=1, space="PSUM"))
        colp = ctx.enter_context(tc.tile_pool(name="colp", bufs=1, space="PSUM"))
        etpool = ctx.enter_context(tc.tile_pool(name="etpool", bufs=4))

        stats = singles.tile([128, S_W], F32)
        raw_own = singles.tile([128, 8, F], F32)
        scaled_sb = [
            singles.tile([128, 2, 8, 128], BF16, name=f"scsb{i}") for i in range(8)
        ]  # chunk-major normalized, one memref per 8-tile sub-batch
        cT = [
            [singles.tile([128, 2048], BF16, name=f"cT{G}_{c}") for c in range(2)]
            for G in range(4)
        ]
        blkT = [singles.tile([128, 8 * 128], BF16, name=f"blkT{c}") for c in range(2)]
        ssb = [singles.tile([128, 8], F32, name=f"ssb{i}") for i in range(8)]
        yB = [singles.tile([128, 8], F32, name=f"yB{i}") for i in range(8)]
        ybb = [singles.tile([128, 8], BF16, name=f"ybb{i}") for i in range(8)]
        e_parts = singles.tile([128, 8, 20], F32)
        ones_b = singles.tile([128, 2], BF16)
        seg = singles.tile([128, 32], BF16)  # seg[p, g] = 1 iff p//4 == g
        seg8 = singles.tile([128, 2, 32], FP8)  # doubled for DR colsums
        magicf = singles.tile([128, NT], F32)
        ostage = ctx.enter_context(tc.tile_pool(name='ostage', bufs=2))

        psum_half = [
            psum.tile([128, CHUNK], F32, name=f"psh{i}") for i in range(2)
        ]  # 2+2 banks, ping-pong as separate memrefs (finer dep tracking)

        nc.vector.memset(magicf[:], float(0x5F3759DF))
        nc.vector.memset(ones_b[:], 1.0)
        segf = singles.tile([128, 32], F32)
        nc.sync.dma_start(segf[:], seg_in.ap())
        nc.vector.tensor_copy(seg[:], segf[:])
        for i in range(2):
            nc.vector.tensor_copy(seg8[:, i, :], seg[:])
        nc.vector.memset(e_parts[:], 0.0)

        cmb_t = cmb.ap().rearrange("(t p) f -> p t f", p=128)

        # ---- DMA loads: own tiles first, then 8-tile sub-batches ascending -
        # (transposes are interleaved into the same SP ring later, in
        # consumption order, to avoid head-of-line blocking)
        for m, r in enumerate(OWN):
            nc.sync.dma_start(raw_own[:, m, :], cmb_t[:, r, :])
        raw_sb = []

        def load_sb(sb):
            rg = rawp.tile([128, 8, F], F32, tag="rawsb")
            nc.sync.dma_start(rg[:], cmb_t[:, 8 * sb : 8 * (sb + 1), :])
            raw_sb.append(rg)

        for sb in range(8):
            load_sb(sb)

        # ---- normalization helpers -----------------------------------------
        def sumsq(raw_view, ss_col, on_act=False):
            sq = scr.tile([128, F], BF16, tag="sq")
            if on_act:
                # ACT is idle during the load/normalize ramp; Square+accum
                # there shortens the DVE-bound ramp
                nc.scalar.activation(
                    sq[:], raw_view, mybir.ActivationFunctionType.Square,
                    accum_out=ss_col,
                )
            else:
                nc.vector.scalar_tensor_tensor(
                    out=sq[:], in0=raw_view, scalar=0.0, in1=raw_view,
                    op0=OP.bypass, op1=OP.mult, accum_out=ss_col,
                )

        def newton(sb):
            """rsqrt of ssb[sb] -> yB[sb] (fp32) and ybb[sb] (bf16)."""
            ss, y = ssb[sb], yB[sb]
            n = 8
            sm = scr.tile([128, n], F32, tag="nm")
            nc.vector.tensor_scalar_max(sm[:], ss[:], EPS2)
            bits_f = scr.tile([128, n], F32, tag="nb")
            nc.vector.tensor_copy(bits_f[:], sm[:].bitcast(U32))
            seed_f = scr.tile([128, n], F32, tag="ns")
            nc.vector.scalar_tensor_tensor(
                out=seed_f[:], in0=bits_f[:], scalar=-0.5, in1=magicf[:, :n],
                op0=OP.mult, op1=OP.add,
            )
            nc.vector.tensor_copy(y[:].bitcast(U32), seed_f[:])
            for _ in range(2):
                t1 = scr.tile([128, n], F32, tag="nr")
                nc.vector.tensor_tensor(t1[:], y[:], y[:], OP.mult)
                t2 = scr.tile([128, n], F32, tag="nr")
                nc.vector.scalar_tensor_tensor(
                    out=t2[:], in0=t1[:], scalar=-0.5, in1=sm[:],
                    op0=OP.mult, op1=OP.mult,
                )
                t3 = scr.tile([128, n], F32, tag="nr")
                nc.vector.tensor_scalar_add(t3[:], t2[:], 1.5)
                nc.vector.tensor_tensor(y[:], y[:], t3[:], OP.mult)
            nc.vector.tensor_copy(ybb[sb][:], y[:])

        def scale_tiles(raw_view, sb):
            """scaled_sb[sb] = raw * yb[t] (bf16, on GpSimd: DVE is the
            pacing engine and the Q7s are otherwise idle)."""
            yv = ybb[sb][:].broadcast_to([128, 8, 128])
            for c in range(2):
                nc.vector.scalar_tensor_tensor(
                    out=scaled_sb[sb][:, c, :, :],
                    in0=raw_view[:, :, 128 * c : 128 * (c + 1)],
                    scalar=1.0, in1=yv, op0=OP.mult, op1=OP.mult,
                )

        # ---- own tiles: normalize, transpose, gf --------------------------
        ss_own = singles.tile([128, 8], F32)
        for m in range(8):
            sumsq(raw_own[:, m, :], ss_own[:, m : m + 1], on_act=True)
        # own norms -> compact newton, then scatter scale per tile
        y_own = singles.tile([128, 8], F32)
        yb_own = singles.tile([128, 8], BF16)
        nc.vector.tensor_scalar_max(ss_own[:], ss_own[:], EPS2)
        bits_f = scr.tile([128, 8], F32, tag="nb")
        nc.vector.tensor_copy(bits_f[:], ss_own[:].bitcast(U32))
        seed_f = scr.tile([128, 8], F32, tag="ns")
        nc.vector.scalar_tensor_tensor(
            out=seed_f[:], in0=bits_f[:], scalar=-0.5, in1=magicf[:, :8],
            op0=OP.mult, op1=OP.add,
        )
        nc.vector.tensor_copy(y_own[:].bitcast(U32), seed_f[:])
        for _ in range(2):
            t1 = scr.tile([128, 8], F32, tag="nr")
            nc.vector.tensor_tensor(t1[:], y_own[:], y_own[:], OP.mult)
            t2 = scr.tile([128, 8], F32, tag="nr")
            nc.vector.scalar_tensor_tensor(
                out=t2[:], in0=t1[:], scalar=-0.5, in1=ss_own[:],
                op0=OP.mult, op1=OP.mult,
            )
            t3 = scr.tile([128, 8], F32, tag="nr")
            nc.vector.tensor_scalar_add(t3[:], t2[:], 1.5)
            nc.vector.tensor_tensor(y_own[:], y_own[:], t3[:], OP.mult)
        nc.vector.tensor_copy(yb_own[:], y_own[:])
        scaled_own = singles.tile([128, 2, 8, 128], BF16)
        yv_own = yb_own[:].broadcast_to([128, 8, 128])
        for c in range(2):
            nc.vector.scalar_tensor_tensor(
                out=scaled_own[:, c, :, :],
                in0=raw_own[:, :, 128 * c : 128 * (c + 1)],
                scalar=1.0, in1=yv_own, op0=OP.mult, op1=OP.mult,
            )
        # seed ss with the own sums-of-squares so each sub-batch's newton and
        # scale cover the own tiles too (bit-identical to the own-phase path)
        for m, r in enumerate(OWN):
            nc.vector.tensor_copy(ssb[r // 8][:, r % 8 : r % 8 + 1],
                                  ss_own[:, m : m + 1])
        # own transposes into blkT (batched: one DMA per K-chunk)
        for c in range(2):
            nc.sync.dma_start_transpose(
                out=blkT[c][:].rearrange("p (t i) -> p t i", i=128),
                in_=scaled_own[:, c, :, :],
            )
        # gf: sum of own scaled rows per K-chunk (tiny ones matmuls)
        for c in range(2):
            for m in range(8):
                nc.tensor.matmul(
                    psum_half[0][:, c : c + 1],
                    scaled_own[:, c, m, :],
                    ones_b[:, 0:1],
                    start=(m == 0), stop=(m == 7),
                )
        nc.vector.tensor_copy(stats[:, S_GF : S_GF + 2], psum_half[0][:, 0:2])
        # ---- normalization + transposes, 8-tile sub-batches (ascending) ----
        for sb in range(8):
            t0 = 8 * sb
            rg = raw_sb[sb]
            for i in range(8):
                t = t0 + i
                if t in OWN:
                    continue
                sumsq(rg[:, i, :], ssb[sb][:, i : i + 1], on_act=(sb < 4))
            with tc.high_priority(offset=40):
                newton(sb)
                scale_tiles(rg[:], sb)
                for c in range(2):
                    nc.sync.dma_start_transpose(
                        out=cT[sb // 2][c][
                            :, 1024 * (sb % 2) : 1024 * (sb % 2 + 1)
                        ].rearrange("p (t i) -> p t i", i=128),
                        in_=scaled_sb[sb][:, c, :, :],
                    )


        # ---- main loop: per column-group, strips -> exp -> colsums ---------
        half = 0
        eslot = [0] * 8  # rowsum slot counter per own tile
        pending = []  # deferred colsum emissions (1-unit lag for PE overlap)

        def emit_mms(m, c0, c1, ph):
            w = c1 - c0
            G, gof = c0 // 2048, c0 % 2048
            for c in range(2):
                lhs = blkT[c][:, 128 * m : 128 * (m + 1)]
                o = 0
                while o < w:
                    oe = min(o + 512, w)
                    nc.tensor.matmul(
                        psum_half[ph][:, o:oe],
                        lhs, cT[G][c][:, gof + o : gof + oe],
                        start=(c == 0), stop=(c == 1),
                    )
                    o = oe

        def flush_pending():
            while pending:
                pending.pop(0)()

        colacc = None
        colacc_first = None

        for G in range(4):
            colacc = colp.tile([128, 2048], F32, tag="colacc")
            colacc_first = [True] * 4  # per 512-chunk of this group

            def colsum_mm(et_ap, c0, c1, kind):
                """Emit colsum matmuls for et (SBUF) covering cols [c0,c1)."""
                dr = kind == "dr"
                lhs = {"dr": seg8[:], "bf": seg[:],
                       "f32r": segf[:].bitcast(mybir.dt.float32r)}[kind]
                c = c0
                while c < c1:
                    q, qoff = divmod(c - 2048 * G, 512)
                    ce = min(c1, 2048 * G + 512 * (q + 1))
                    first = colacc_first[q]
                    colacc_first[q] = False
                    out_ap = colacc[0:32, 512 * q + qoff : 512 * q + qoff + (ce - c)]
                    in_ap = et_ap[:, c - c0 : ce - c0] if not dr else \
                        et_ap[:, :, c - c0 : ce - c0]
                    nc.tensor.matmul(
                        out_ap,
                        lhs,
                        in_ap,
                        start=first, stop=False,
                        perf_mode=mybir.MatmulPerfMode.DoubleRow if dr else None,
                        skip_group_check=True,
                    )
                    c = ce

            for unit in WORK[G]:
                if unit[0] == "single":
                    _, m, c0, c1, diag = unit
                    w = c1 - c0
                    ph = half
                    half ^= 1
                    emit_mms(m, c0, c1, ph)
                    et = etpool.tile([128, CHUNK], BF16, tag="etb")
                    sl = eslot[m]
                    eslot[m] += 1
                    nc.scalar.activation(
                        et[:, :w], psum_half[ph][:, :w],
                        mybir.ActivationFunctionType.Exp,
                        bias=0.0, scale=INV_T,
                        accum_out=e_parts[:, m, sl : sl + 1],
                    )
                    cs0 = c0 + 128 if diag else c0
                    if cs0 < c1:
                        pending.append(
                            lambda et=et, cs0=cs0, c0=c0, c1=c1: colsum_mm(
                                et[:, cs0 - c0 :], cs0, c1, "bf"
                            )
                        )
                elif unit[0] == "dpair":
                    _, m1, m2, c0, c1 = unit
                    w = c1 - c0
                    etds = []
                    for m in (m1, m2):
                        ph = half
                        half ^= 1
                        emit_mms(m, c0, c1, ph)
                        etd = etpool.tile([128, CHUNK], F32, tag="etd")
                        sl = eslot[m]
                        eslot[m] += 1
                        with tc.high_priority(offset=1500):
                            nc.vector.tensor_scalar(
                                out=etd[:, :w].bitcast(U32),
                                in0=psum_half[ph][:, :w],
                                scalar1=SCHRAUD_A, scalar2=SCHRAUD_B,
                                op0=OP.mult, op1=OP.add,
                            )
                            nc.vector.tensor_reduce(
                                e_parts[:, m, sl : sl + 1], etd[:, :w],
                                AX.X, OP.add,
                            )
                        etds.append(etd)
                    pending.append(
                        lambda etds=etds, c0=c0, c1=c1, w=w: [
                            colsum_mm(
                                e[:, :w].bitcast(mybir.dt.float32r),
                                c0, c1, "f32r",
                            )
                            for e in etds
                        ]
                    )
                else:
                    _, m1, m2, c0, c1 = unit
                    w = c1 - c0
                    etp = etpool.tile([128, 2, CHUNK], FP8, tag="etp")
                    for pi, m in enumerate((m1, m2)):
                        ph = half
                        half ^= 1
                        emit_mms(m, c0, c1, ph)
                        sl = eslot[m]
                        eslot[m] += 1
                        nc.scalar.activation(
                            etp[:, pi, :w], psum_half[ph][:, :w],
                            mybir.ActivationFunctionType.Exp,
                            bias=0.0, scale=INV_T,
                            accum_out=e_parts[:, m, sl : sl + 1],
                        )
                    pending.append(
                        lambda etp=etp, c0=c0, c1=c1: colsum_mm(
                            etp[:, :, : c1 - c0], c0, c1, "dr"
                        )
                    )
                # lagged colsum emission: keep PE two units ahead so the
                # colsum matmul (which waits on an earlier exp) never blocks
                # the next unit's matmuls in PE program order
                while len(pending) > 2:
                    pending.pop(0)()
            flush_pending()
            # drain this group's colsum accumulators to SBUF, then to DRAM
            stg = ostage.tile([128, 2048], F32, tag="ostg")
            nc.vector.tensor_copy(stg[0:32, :], colacc[0:32, :])
            nc.sync.dma_start(ocols.ap()[G, :, :], stg[0:32, :])

        # d_i and pos-pair dots from own scaled tiles
        for m in range(8):
            sq = scr.tile([128, 2, 128], BF16, tag="sqd")
            nc.vector.scalar_tensor_tensor(
                out=sq[:], in0=scaled_own[:, :, m, :], scalar=0.0,
                in1=scaled_own[:, :, m, :], op0=OP.bypass, op1=OP.mult,
                accum_out=stats[:, S_D + m : S_D + m + 1],
            )
        for a in range(4):
            sq = scr.tile([128, 2, 128], BF16, tag="sqp")
            nc.vector.scalar_tensor_tensor(
                out=sq[:], in0=scaled_own[:, :, a, :], scalar=0.0,
                in1=scaled_own[:, :, a + 4, :], op0=OP.bypass, op1=OP.mult,
                accum_out=stats[:, S_POS + a : S_POS + a + 1],
            )

        # ---- finalize stats -------------------------------------------------
        for m in range(8):
            nc.vector.tensor_reduce(
                stats[:, S_E + m : S_E + m + 1], e_parts[:, m, :], AX.X, OP.add
            )
        nc.sync.dma_start(out.ap(), stats[:])


_NC_CACHE = None


def _get_nc():
    global _NC_CACHE
    if _NC_CACHE is None:
        _NC_CACHE = _build_kernel()
    return _NC_CACHE


def make_in_maps(first, second):
    f = np.ascontiguousarray(first, dtype=np.float32)
    s = np.ascontiguousarray(second, dtype=np.float32)
    cmb = np.concatenate([f, s], axis=0)
    seg_host = np.zeros((128, 32), dtype=np.float32)
    seg_host[np.arange(128), np.arange(128) // 4] = 1.0
    return [
        {
            "cmb": np.ascontiguousarray(np.roll(cmb, -128 * k, axis=0)),
            "seg_in": seg_host,
        }
        for k in range(N_CORES)
    ]


def combine_outputs(outs):
    """outs: list of 8 dicts with 'out' [128, S_W] and 'ocols' [4, 8192]."""
    E = np.zeros(TWO_B, dtype=np.float64)  # global-row exp sums
    d = np.zeros(TWO_B, dtype=np.float64)
    gf = np.zeros((128, 2), dtype=np.float64)
    pos_tot = 0.0
    for k in range(N_CORES):
        st = np.asarray(outs[k]["out"], dtype=np.float64)
        oc = np.asarray(outs[k]["ocols"], dtype=np.float64)
        # rowsums + d for own canonical tiles -> global rows
        for m, r in enumerate(OWN):
            rows = (np.arange(128) + 128 * (r + k)) % TWO_B  # canonical->global
            E[rows] += st[:, S_E + m]
            d[rows] = st[:, S_D + m]
        pos_tot += st[:, S_POS : S_POS + 4].sum()
        gf += st[:, S_GF : S_GF + 2]
        # colsums: oc[G, g, n] = partial sum (rows 4g..4g+4) for canonical
        # col 2048G + n
        cc = oc.sum(axis=1)  # [G, n]
        can_cols = 2048 * np.arange(4)[:, None] + np.arange(2048)[None, :]
        gcols = (can_cols + 128 * k) % TWO_B
        np.add.at(E, gcols.ravel(), cc.ravel())
    E_excl = E - np.exp(INV_T * d)
    lse = np.log(E_excl)
    raw_excl = (float((gf * gf).sum()) - d.sum()) * INV_T
    neg = raw_excl - (TWO_B - 1) * lse.sum()
    loss = -pos_tot * INV_T / B + neg / (4.0 * B * B)
    return np.asarray(loss, dtype=np.float32)


def kernel(first_transformed, second_transformed):
    nc = _get_nc()
    in_maps = make_in_maps(first_transformed, second_transformed)
    res = run_bass_kernel_spmd(nc, in_maps, core_ids=list(range(N_CORES)))
    return combine_outputs(res.results)
